# revision 52
# baseline (speedup 1.0000x reference)
"""Trainium2 Bass kernel for LocalFeatureAggregation (gnn_message_passing).

Sharding: data-parallel over points. Each of the 8 cores gets 12500 points
(padded to 12544 = 98 tiles of 128). Neighbor geometry (rel/dist/orig/nbr,
the gather-dependent part) is computed on host per the sharding hint and
shipped as bf16. MLP weights replicated.

Math decomposition (validated vs reference):
  lse+pool(x, f): z = [enc | f], softmax over channels, sum over K.
    Split into enc-half / f-half:
      E=exp(enc); s_k = sum_c E; Ef=exp(f); Sf=sum_c Ef; r_k = 1/(s_k+Sf)
      pool_enc = sum_k enc*E*r_k ; pool_f = (f*Ef) * sum_k r_k
"""
import math
import os
import numpy as np
import ml_dtypes

import concourse.bass as bass
import concourse.bacc as bacc
import concourse.mybir as mybir
import concourse.tile as tile
from concourse.bass_utils import run_bass_kernel_spmd
from concourse.masks import make_identity

AF = mybir.ActivationFunctionType
ALU = mybir.AluOpType
AX = mybir.AxisListType
F32 = mybir.dt.float32
BF16 = mybir.dt.bfloat16
BF = ml_dtypes.bfloat16

N_FULL = 100000
NCORES = 8
NSH = N_FULL // NCORES      # 12500
T = 128                     # points per tile
K = 16                      # neighbors
S = 7                       # tiles per supertile
D_IN = 128
D_OUT = 256


def _ceil_to(x, m):
    return ((x + m - 1) // m) * m


def _bcast_last(ap2d, last):
    """[P, F] AP -> [P, F, last] AP with step-0 trailing dim."""
    a = ap2d.rearrange("p (f a) -> p f a", a=1)
    return a.to_broadcast([ap2d.shape[0], ap2d.shape[1], last])


def build(nsh_pad):
    """Emit the bass program for one core processing nsh_pad points."""
    assert nsh_pad % (T * S) == 0
    nt = nsh_pad // T           # tiles
    nst = nt // S               # supertiles
    FW = S * T                  # feature-stage width (896)
    CH = [(0, 512), (512, 384)]

    nc = bacc.Bacc(trn_type="TRN2")

    # ---------------- DRAM tensors ----------------
    featT = nc.dram_tensor("featT", [D_IN, nsh_pad], BF16, kind="ExternalInput")
    geoT = nc.dram_tensor("geoT", [10, K, nsh_pad], BF16, kind="ExternalInput")
    out_d = nc.dram_tensor("out", [nt, 4 * T, T], BF16, kind="ExternalOutput")

    def din(name, shape, dt):
        return nc.dram_tensor(name, shape, dt, kind="ExternalInput")

    wd = {}
    for ln in ("l1", "l2"):
        wd[ln + "_W1a"] = din(ln + "_W1a", [10, 65], BF16)
        wd[ln + "_b1a"] = din(ln + "_b1a", [65, 1], F32)
        wd[ln + "_W2a"] = din(ln + "_W2a", [65, 128], BF16)
    wd["m1_W1"] = din("m1_W1", [128, 128], BF16)
    wd["m1_W2"] = din("m1_W2", [128, 128], BF16)
    wd["m1_b1"] = din("m1_b1", [128, 1], F32)
    wd["m1_b2"] = din("m1_b2", [128, 1], F32)
    wd["p1_W1"] = din("p1_W1", [128, 512], BF16)
    wd["p1_b1"] = din("p1_b1", [128, 2], F32)
    wd["p1_W2"] = din("p1_W2", [128, 256], BF16)
    wd["p1_b2"] = din("p1_b2", [128, 1], F32)
    wd["p2_W1"] = din("p2_W1", [128, 512], BF16)
    wd["p2_b1"] = din("p2_b1", [128, 2], F32)
    wd["p2_W2"] = din("p2_W2", [128, 512], BF16)
    wd["p2_b2"] = din("p2_b2", [128, 2], F32)
    wd["m2_W1"] = din("m2_W1", [128, 768], BF16)     # [256,384]
    wd["m2_b1"] = din("m2_b1", [128, 3], F32)
    wd["m2_W2"] = din("m2_W2", [128, 1536], BF16)    # [384,512]
    wd["m3_W1"] = din("m3_W1", [128, 256], BF16)     # [128,256]
    wd["m3_b1"] = din("m3_b1", [128, 2], F32)
    wd["m3_W2"] = din("m3_W2", [128, 1024], BF16)    # [256,512]
    wd["b2s"] = din("b2s", [128, 4], F32)            # bpack(m2_b2+m3_b2)

    with tile.TileContext(nc) as tc:
        import contextlib
        with contextlib.ExitStack() as ctx:
            cst = ctx.enter_context(tc.tile_pool(name="cst", bufs=1))
            geo_p = ctx.enter_context(tc.tile_pool(name="geo", bufs=7))
            enc_p = ctx.enter_context(tc.tile_pool(name="enc", bufs=4))
            pool_p = ctx.enter_context(tc.tile_pool(name="pool", bufs=2))
            feat_p = ctx.enter_context(tc.tile_pool(name="feat", bufs=2))
            ps_z = ctx.enter_context(
                tc.tile_pool(name="psz", bufs=2, space="PSUM"))
            ps_h = ctx.enter_context(
                tc.tile_pool(name="psh", bufs=1, space="PSUM"))
            ps_m = ctx.enter_context(
                tc.tile_pool(name="psm", bufs=2, space="PSUM"))

            # ---------- constants ----------
            w_sb = {}
            for name, dr in wd.items():
                t_ = cst.tile(list(dr.shape), dr.dtype, name="w_" + name)
                nc.sync.dma_start(out=t_[:], in_=dr[:])
                w_sb[name] = t_
            ident = cst.tile([128, 128], BF16, name="ident")
            make_identity(nc, ident[:])

            def fmlp(rhs_tiles, wkey, bkey, nci, ncout, func, out_t,
                     leaky=False, extra=None, defer=None, chunk_sel=None):
                """Channel-major MLP layer over the supertile.
                rhs_tiles: list of nci tiles [128, FW]; out_t [128, nco_blk, FW].
                extra: (rhs_tiles2, wkey2, nci2) accumulated into same psum.
                defer: if a list, append one emission closure per
                (co, chunk) block instead of emitting now."""
                w = w_sb[wkey]
                b = w_sb[bkey]
                nblk = ncout // 128

                def emit_block(co, c0, cw):
                    ps = ps_m.tile([128, 512], F32, tag="misc", name="ps_f")
                    tot = nci + (extra[2] if extra else 0)
                    i = 0
                    for ci in range(nci):
                        nc.tensor.matmul(
                            out=ps[:, :cw],
                            lhsT=w[:, ci * ncout + co * 128:
                                   ci * ncout + (co + 1) * 128],
                            rhs=rhs_tiles[ci][:, c0:c0 + cw],
                            start=(i == 0),
                            stop=(i == tot - 1))
                        i += 1
                    if extra:
                        rhs2, wkey2, nci2 = extra
                        w2 = w_sb[wkey2]
                        for ci in range(nci2):
                            nc.tensor.matmul(
                                out=ps[:, :cw],
                                lhsT=w2[:, ci * ncout + co * 128:
                                        ci * ncout + (co + 1) * 128],
                                rhs=rhs2[ci][:, c0:c0 + cw],
                                start=(i == 0),
                                stop=(i == tot - 1))
                            i += 1
                    if leaky:
                        nc.scalar.activation(
                            out=out_t[:, co, c0:c0 + cw], in_=ps[:, :cw],
                            func=AF.Prelu, bias=b[:, co:co + 1],
                            alpha=0.01)
                    else:
                        nc.scalar.activation(
                            out=out_t[:, co, c0:c0 + cw], in_=ps[:, :cw],
                            func=func, bias=b[:, co:co + 1])

                for ci_, (c0, cw) in enumerate(CH):
                    if chunk_sel is not None and ci_ not in chunk_sel:
                        continue
                    for co in range(nblk):
                        if defer is None:
                            emit_block(co, c0, cw)
                        else:
                            defer.append(
                                lambda co=co, c0=c0, cw=cw:
                                emit_block(co, c0, cw))

            def fhalf_alloc():
                Ef = pool_p.tile([128, S, T], BF16, tag="Ef", name="Ef")
                Sf = pool_p.tile([128, S], F32, tag="Sf", name="Sf")
                g = pool_p.tile([128, S, T], BF16, tag="g", name="g")
                return None, Ef, Sf, g

            def fhalf_wave(fh, xcm, t0, t1):
                """f-half for tiles [t0, t1) of a round."""
                _, Ef, Sf, g = fh
                w = (t1 - t0) * T
                fTp = ps_m.tile([128, w], BF16, tag="misc", name="fTp")
                for t in range(t0, t1):
                    nc.tensor.transpose(
                        out=fTp[:, (t - t0) * T:(t - t0 + 1) * T],
                        in_=xcm[:, t * T:(t + 1) * T], identity=ident[:])
                nc.scalar.activation(
                    out=Ef[:, t0:t1, :].rearrange("p s t -> p (s t)"),
                    in_=fTp[:], func=AF.Exp)
                nc.vector.reduce_sum(out=Sf[:, t0:t1], in_=Ef[:, t0:t1, :],
                                     axis=AX.X)
                nc.vector.tensor_mul(
                    out=g[:, t0:t1, :].rearrange("p s t -> p (s t)"),
                    in0=fTp[:],
                    in1=Ef[:, t0:t1, :].rearrange("p s t -> p (s t)"))

            def enc_geo(geo_t, lname, s_slot):
                """x-independent per-tile encoder work: h, E, M, s."""
                w1a = w_sb[lname + "_W1a"]
                b1a = w_sb[lname + "_b1a"]
                w2a = w_sb[lname + "_W2a"]
                # L1: relu(W1a.T @ geo) -> h [65, K, T] bf16 (row 64 == 1)
                h = enc_p.tile([65, K, T], BF16, tag="h", name="h")
                for c in range(2):
                    ph = ps_h.tile([65, 8, T], F32, tag="ph", name="ph")
                    for c2 in range(2):
                        nc.tensor.matmul(
                            out=ph[:, 4 * c2:4 * c2 + 4, :], lhsT=w1a[:],
                            rhs=geo_t[0:10, 8 * c + 4 * c2:
                                      8 * c + 4 * c2 + 4, :])
                    nc.scalar.activation(
                        out=h[:, 8 * c:8 * c + 8, :], in_=ph[:],
                        func=AF.Relu, bias=b1a[:, 0:1])
                # L2 row-major: z_k = h_k.T @ W2a -> psum [pt, c]
                z = ps_z.tile([128, K, T], F32, tag="z", name="z", bufs=1)
                for k in range(K):
                    nc.tensor.matmul(
                        out=z[:, k, :], lhsT=h[:, k, :], rhs=w2a[:])
                E = enc_p.tile([128, K, T], BF16, tag="E", name="E", bufs=2)
                M = enc_p.tile([128, K, T], BF16, tag="M", name="M", bufs=8)
                nc.scalar.activation(out=E[:], in_=z[:], func=AF.Exp)
                nc.vector.tensor_mul(out=M[:], in0=E[:], in1=z[:])
                # s[pt, k] = sum_c E -> batched per-round tile slot
                with nc.allow_low_precision("bf16 softmax denom"):
                    nc.vector.reduce_sum(out=s_slot, in_=E[:], axis=AX.X)
                return M

            def wave_r(s_all, Sf_all, g_all, t0, t1):
                """Batched r/sr/gsr for tiles [t0, t1): r_all slice + gsr."""
                w = t1 - t0
                r_all = pool_p.tile([128, S, K], F32, tag="r", name="r_all",
                                    bufs=2) if t0 == 0 else None
                if t0 == 0:
                    wave_r.cur = r_all
                r_all = wave_r.cur
                den = r_all[:, t0:t1, :]
                nc.vector.tensor_tensor(
                    out=den, in0=s_all[:, t0:t1, :],
                    in1=_bcast_last(Sf_all[:, t0:t1], K), op=ALU.add)
                nc.vector.reciprocal(
                    out=den.rearrange("p s k -> p (s k)"),
                    in_=den.rearrange("p s k -> p (s k)"))
                sr_w = pool_p.tile([128, S], F32, tag="sr", name="sr_w",
                                   bufs=2) if t0 == 0 else None
                if t0 == 0:
                    wave_r.sr = sr_w
                sr_w = wave_r.sr
                nc.vector.reduce_sum(out=sr_w[:, t0:t1],
                                     in_=r_all[:, t0:t1, :], axis=AX.X)
                gsr = pool_p.tile([128, S, T], BF16, tag="gsr", name="gsr",
                                  bufs=2) if t0 == 0 else None
                if t0 == 0:
                    wave_r.gsr = gsr
                gsr = wave_r.gsr
                nc.gpsimd.tensor_tensor(
                    out=gsr[:, t0:t1, :], in0=g_all[:, t0:t1, :],
                    in1=_bcast_last(sr_w[:, t0:t1], T), op=ALU.mult)
                return r_all, gsr

            def enc_tail(M, r_all, t_in_s):
                """x-dependent pool tail for one tile (enc half)."""
                # ctb = M * r_k (gpsimd); k-sum via tree adds
                ctb = enc_p.tile([128, K, T], BF16, tag="ct", name="ctb")
                nc.gpsimd.tensor_tensor(
                    out=ctb[:], in0=M[:],
                    in1=_bcast_last(r_all[:, t_in_s, :], T),
                    op=ALU.mult)
                pr = pool_p.tile([128, T], BF16, tag="pr", name="pr",
                                 bufs=5)
                nc.vector.tensor_add(
                    out=ctb[:, 0:8, :], in0=ctb[:, 0:8, :],
                    in1=ctb[:, 8:16, :])
                for hw_ in (4, 2):
                    nc.gpsimd.tensor_add(
                        out=ctb[:, 0:hw_, :], in0=ctb[:, 0:hw_, :],
                        in1=ctb[:, hw_:2 * hw_, :])
                nc.gpsimd.tensor_add(out=pr[:], in0=ctb[:, 0, :],
                                     in1=ctb[:, 1, :])
                return pr

            def pool_transpose(pr, gsr, t_in_s, pcm):
                # transpose pool rows back to channel-major (deferred so it
                # does not gate the next round's PE work); one fused copy
                c0 = t_in_s * T
                pt2 = ps_m.tile([128, 2 * T], BF16, tag="misc", name="pt2")
                nc.tensor.transpose(
                    out=pt2[:, 0:T], in_=pr[:], identity=ident[:])
                nc.tensor.transpose(
                    out=pt2[:, T:2 * T], in_=gsr[:, t_in_s, :],
                    identity=ident[:])
                nc.scalar.copy(
                    out=pcm[:, :, c0:c0 + T],
                    in_=pt2[:].rearrange("p (h t) -> p h t", h=2))

            # ================= main loop (wave-pipelined rounds) ====
            # Rounds r: even = enc1(st), odd = enc2(st). Each round runs in
            # two waves (tiles 0-3, 4-6). A wave's pool results feed the
            # p-MLP chunk for that wave immediately, so the next round's
            # f-half (and hence its tails) start before this round ends.
            pending = []

            def prelude(st):
                # ft DMA + m1 MLP for supertile st -> (ft, x1)
                g0 = st * S
                ft = feat_p.tile([128, 1, FW], BF16, tag="ft", name="ft",
                                 bufs=3)
                nc.sync.dma_start(
                    out=ft[:, 0, :], in_=featT[:, g0 * T:(g0 + S) * T])
                m1h = feat_p.tile([128, 1, FW], BF16, tag="m1h", name="m1h", bufs=1)
                fmlp([ft[:, 0, :]], "m1_W1", "m1_b1", 1, 128, AF.Relu, m1h)
                x1 = feat_p.tile([128, 1, FW], BF16, tag="x1", name="x1")
                fmlp([m1h[:, 0, :]], "m1_W2", "m1_b2", 1, 128, AF.Identity,
                     x1)
                return ft, x1[:, 0, :]

            nrounds = 2 * nst
            all_geos = {}

            def geo_dma(st, t):
                g = st * S + t
                geo_t = geo_p.tile([10, K, T], BF16, tag="geo", name="geo_t")
                nc.sync.dma_start(out=geo_t[:],
                                  in_=geoT[:, :, g * T:(g + 1) * T])
                all_geos[(st, t)] = geo_t

            def geo_side(r, t, s_all):
                st, ln = r // 2, ("l1" if r % 2 == 0 else "l2")
                if ln == "l1":
                    geo_dma(st, t)
                return enc_geo(all_geos[(st, t)], ln, s_all[:, t, :])

            WAVES = [(0, 4), (4, S)]
            ft_x1 = prelude(0)
            s_cur = pool_p.tile([128, S, K], BF16, tag="sal", name="s_cur",
                                bufs=2)
            cur = [geo_side(0, t, s_cur) for t in range(S)]
            fh = fhalf_alloc()
            rg = [None, None]
            for wi, (t0, t1) in enumerate(WAVES):
                fhalf_wave(fh, ft_x1[1], t0, t1)
                rg[wi] = wave_r(s_cur, fh[2], fh[3], t0, t1)
            ctx_st = {}
            for r in range(nrounds):
                st = r // 2
                even = (r % 2 == 0)
                if even:
                    ft, x1 = ft_x1
                    ctx_st = {"ft": ft}
                    pcm = feat_p.tile([128, 2, FW], BF16, tag="p1cm",
                                      name="pcm1")
                else:
                    pcm = feat_p.tile([128, 2, FW], BF16, tag="p2cm",
                                      name="pcm2")
                fhn = None
                prs = []
                nxt = []
                rg_cur, rg = rg, [None, None]
                if r + 1 < nrounds:
                    s_cur = pool_p.tile([128, S, K], BF16, tag="sal",
                                        name="s_cur", bufs=2)
                for wi, (t0, t1) in enumerate(WAVES):
                    r_all, gsr = rg_cur[wi]
                    for t in range(t0, t1):
                        prs.append(enc_tail(cur[t], r_all, t))
                        if r + 1 < nrounds:
                            nxt.append(geo_side(r + 1, t, s_cur))
                        if pending:
                            pending.pop(0)()
                        if pending:
                            pending.pop(0)()
                    for t in range(t0, t1):
                        pool_transpose(prs[t], gsr, t, pcm)
                    # p-MLP chunk for this wave -> x_{r+1} chunk
                    if even:
                        if wi == 0:
                            p1h = feat_p.tile([128, 2, FW], BF16, tag="p1h",
                                              name="p1h")
                            x2t = feat_p.tile([128, 1, FW], BF16, tag="x2",
                                              name="x2t")
                        fmlp([pcm[:, 0, :], pcm[:, 1, :]], "p1_W1", "p1_b1",
                             2, 256, AF.Relu, p1h, chunk_sel=[wi])
                        fmlp([p1h[:, 0, :], p1h[:, 1, :]], "p1_W2", "p1_b2",
                             2, 128, AF.Identity, x2t, chunk_sel=[wi])
                        xn = x2t[:, 0, :]
                    else:
                        if wi == 0:
                            p2h = feat_p.tile([128, 2, FW], BF16, tag="p2h",
                                              name="p2h")
                            x3 = feat_p.tile([128, 2, FW], BF16, tag="x3",
                                             name="x3")
                        fmlp([pcm[:, 0, :], pcm[:, 1, :]], "p2_W1", "p2_b1",
                             2, 256, AF.Relu, p2h, chunk_sel=[wi])
                        fmlp([p2h[:, 0, :], p2h[:, 1, :]], "p2_W2", "p2_b2",
                             2, 256, AF.Identity, x3, chunk_sel=[wi])
                        xn = ctx_st.get("x1n")
                    # next round's f-half + r-chain for this wave
                    if r + 1 < nrounds:
                        if wi == 0:
                            fhn = fhalf_alloc()
                        fhalf_wave(fhn, xn, t0, t1)
                        rg[wi] = wave_r(s_cur, fhn[2], fhn[3], t0, t1)
                cur = nxt
                fh = fhn
                if even:
                    ctx_st["x2"] = x2t[:, 0, :]
                    if st + 1 < nst:
                        ft_x1 = prelude(st + 1)
                        ctx_st["x1n"] = ft_x1[1]
                else:
                    g0 = st * S
                    m2h = feat_p.tile([128, 3, FW], BF16, tag="m2h",
                                      name="m2h")
                    fmlp([x3[:, 0, :], x3[:, 1, :]], "m2_W1", "m2_b1", 2,
                         384, AF.Relu, m2h, defer=pending)
                    m3h = feat_p.tile([128, 2, FW], BF16, tag="m3h",
                                      name="m3h")
                    fmlp([ctx_st["ft"][:, 0, :]], "m3_W1", "m3_b1", 1, 256,
                         AF.Relu, m3h, defer=pending)
                    osb = feat_p.tile([128, 4, FW], BF16, tag="osb",
                                      name="osb")
                    fmlp([m2h[:, 0, :], m2h[:, 1, :], m2h[:, 2, :]],
                         "m2_W2", "b2s", 3, 512, None, osb, leaky=True,
                         extra=([m3h[:, 0, :], m3h[:, 1, :]], "m3_W2", 2),
                         defer=pending)

                    def emit_out(osb=osb, g0=g0):
                        for b in range(4):
                            nc.sync.dma_start(
                                out=out_d[g0:g0 + S, b * T:(b + 1) * T, :]
                                .rearrange("t c p -> c t p"),
                                in_=osb[:, b, :].rearrange(
                                    "c (t p) -> c t p", t=S))
                    pending.append(emit_out)
            while pending:
                pending.pop(0)()

    nc.finalize()
    return nc


_BUILD_CACHE = {}


def _get_prog(nsh_pad):
    if nsh_pad not in _BUILD_CACHE:
        _BUILD_CACHE[nsh_pad] = build(nsh_pad)
    return _BUILD_CACHE[nsh_pad]


def _prep_weights(i):
    """Host-side weight packing -> dict of arrays (shared across cores)."""
    o = {}

    def blkpack(W, cout):
        cin = W.shape[0]
        nci = cin // 128
        return np.concatenate([W[ci * 128:(ci + 1) * 128, :]
                               for ci in range(nci)], axis=1)

    def bpack(b):
        nblk = b.shape[0] // 128
        return np.ascontiguousarray(b.reshape(nblk, 128).T)

    for ln in ("l1", "l2"):
        W1, b1 = i[ln + "_W1"], i[ln + "_b1"]
        W2, b2 = i[ln + "_W2"], i[ln + "_b2"]
        W1a = np.concatenate([W1, np.zeros((10, 1), np.float32)], 1)
        W1a = W1a[[6, 7, 8, 9, 0, 1, 2, 3, 4, 5], :]   # [rel,dist,orig,nbr]
        o[ln + "_W1a"] = W1a.astype(BF)
        o[ln + "_b1a"] = np.concatenate(
            [b1, np.ones(1, np.float32)]).reshape(65, 1).astype(np.float32)
        o[ln + "_W2a"] = np.concatenate(
            [W2, b2[None, :]], 0).astype(BF)
    o["m1_W1"] = i["m1_W1"].astype(BF)
    o["m1_W2"] = i["m1_W2"].astype(BF)
    o["m1_b1"] = i["m1_b1"].reshape(128, 1).astype(np.float32)
    o["m1_b2"] = i["m1_b2"].reshape(128, 1).astype(np.float32)
    o["p1_W1"] = blkpack(i["p1_W1"], 256).astype(BF)
    o["p1_b1"] = bpack(i["p1_b1"]).astype(np.float32)
    o["p1_W2"] = blkpack(i["p1_W2"], 128).astype(BF)
    o["p1_b2"] = i["p1_b2"].reshape(128, 1).astype(np.float32)
    o["p2_W1"] = blkpack(i["p2_W1"], 256).astype(BF)
    o["p2_b1"] = bpack(i["p2_b1"]).astype(np.float32)
    o["p2_W2"] = blkpack(i["p2_W2"], 256).astype(BF)
    o["p2_b2"] = bpack(i["p2_b2"]).astype(np.float32)
    o["m2_W1"] = blkpack(i["m2_W1"], 384).astype(BF)
    o["m2_b1"] = bpack(i["m2_b1"]).astype(np.float32)
    o["m2_W2"] = blkpack(i["m2_W2"], 512).astype(BF)
    o["m3_W1"] = blkpack(i["m3_W1"], 256).astype(BF)
    o["m3_b1"] = bpack(i["m3_b1"]).astype(np.float32)
    o["m3_W2"] = blkpack(i["m3_W2"], 512).astype(BF)
    o["b2s"] = bpack(i["m2_b2"] + i["m3_b2"]).astype(np.float32)
    return o


def _prep_core(coords, features, neighbor_idx, c0, c1, nsh_pad):
    nsh = c1 - c0
    pad = nsh_pad - nsh
    feat = features[c0:c1]
    if pad:
        feat = np.concatenate(
            [feat, np.zeros((pad, feat.shape[1]), np.float32)], 0)
    featT = np.ascontiguousarray(feat.T).astype(BF)
    # geometry rows: [rel(3), dist(1), orig(3), nbr(3)] -> [10, K, nsh_pad]
    cs = coords[c0:c1]                               # (nsh, 3)
    nbr = coords[neighbor_idx[c0:c1]]                # (nsh, K, 3)
    orig = np.broadcast_to(cs[:, None, :], nbr.shape)
    rel = orig - nbr
    dist = np.sqrt(np.sum(rel * rel, axis=-1, keepdims=True))
    geo = np.concatenate([rel, dist, orig, nbr], axis=-1)  # (nsh, K, 10)
    if pad:
        geo = np.concatenate(
            [geo, np.zeros((pad, K, 10), np.float32)], 0)
    geoT = np.ascontiguousarray(np.transpose(geo, (2, 1, 0))).astype(BF)
    return {"featT": featT, "geoT": geoT}


def prepare_in_maps(inputs, nsh_pad):
    coords = np.asarray(inputs["coords"], np.float32)
    features = np.asarray(inputs["features"], np.float32)
    idx = np.asarray(inputs["neighbor_idx"])
    wmaps = _prep_weights({k: np.asarray(v, np.float32)
                           for k, v in inputs.items()
                           if k not in ("coords", "features", "neighbor_idx")})
    in_maps = []
    for c in range(NCORES):
        m = dict(wmaps)
        m.update(_prep_core(coords, features, idx,
                            c * NSH, (c + 1) * NSH, nsh_pad))
        in_maps.append(m)
    return in_maps


def assemble_out(results, nsh_pad):
    outs = []
    for c in range(NCORES):
        r = np.asarray(results[c]["out"]).astype(np.float32)  # [nt,512,T]
        r = np.transpose(r, (0, 2, 1)).reshape(nsh_pad, 512)[:NSH]
        outs.append(r)
    return np.ascontiguousarray(np.concatenate(outs, 0))


LAST_RES = None


def kernel(**inputs):
    global LAST_RES
    nsh_pad = _ceil_to(NSH, T * S)               # 12544
    nc = _get_prog(nsh_pad)
    in_maps = prepare_in_maps(inputs, nsh_pad)
    trace = bool(os.environ.get("KERNEL_TRACE"))
    res = run_bass_kernel_spmd(nc, in_maps, core_ids=list(range(NCORES)),
                               trace=trace)
    LAST_RES = res
    return assemble_out(res.results, nsh_pad)


if __name__ == "__main__":
    rng = np.random.default_rng(0)
    inp = {
        "coords": rng.standard_normal((N_FULL, 3)).astype(np.float32),
        "features": rng.standard_normal((N_FULL, 128)).astype(np.float32),
        "neighbor_idx": rng.integers(0, N_FULL, (N_FULL, 16), dtype=np.int32),
    }
    for nm, ci, ch, co in [("m1", 128, 128, 128), ("m2", 256, 384, 512),
                           ("m3", 128, 256, 512), ("l1", 10, 64, 128),
                           ("l2", 10, 64, 128), ("p1", 256, 256, 128),
                           ("p2", 256, 256, 256)]:
        inp[nm + "_W1"] = rng.standard_normal((ci, ch)).astype(np.float32)
        inp[nm + "_b1"] = rng.standard_normal(ch).astype(np.float32)
        inp[nm + "_W2"] = rng.standard_normal((ch, co)).astype(np.float32)
        inp[nm + "_b2"] = rng.standard_normal(co).astype(np.float32)
    out = kernel(**inp)
    print("out", out.shape, out.dtype)


# revision 53
# speedup vs baseline: 1.0913x; 1.0913x over previous
"""Trainium2 Bass kernel for LocalFeatureAggregation (gnn_message_passing).

Sharding: data-parallel over points. Each of the 8 cores gets 12500 points
(padded to 12544 = 98 tiles of 128). Neighbor geometry (rel/dist/orig/nbr,
the gather-dependent part) is computed on host per the sharding hint and
shipped as bf16. MLP weights replicated.

Math decomposition (validated vs reference):
  lse+pool(x, f): z = [enc | f], softmax over channels, sum over K.
    Split into enc-half / f-half:
      E=exp(enc); s_k = sum_c E; Ef=exp(f); Sf=sum_c Ef; r_k = 1/(s_k+Sf)
      pool_enc = sum_k enc*E*r_k ; pool_f = (f*Ef) * sum_k r_k
"""
import math
import os
import numpy as np
import ml_dtypes

import concourse.bass as bass
import concourse.bacc as bacc
import concourse.mybir as mybir
import concourse.tile as tile
from concourse.bass_utils import run_bass_kernel_spmd
from concourse.masks import make_identity

AF = mybir.ActivationFunctionType
ALU = mybir.AluOpType
AX = mybir.AxisListType
F32 = mybir.dt.float32
BF16 = mybir.dt.bfloat16
BF = ml_dtypes.bfloat16

N_FULL = 100000
NCORES = 8
NSH = N_FULL // NCORES      # 12500
T = 128                     # points per tile
K = 16                      # neighbors
S = 7                       # tiles per supertile
D_IN = 128
D_OUT = 256


def _ceil_to(x, m):
    return ((x + m - 1) // m) * m


def _bcast_last(ap2d, last):
    """[P, F] AP -> [P, F, last] AP with step-0 trailing dim."""
    a = ap2d.rearrange("p (f a) -> p f a", a=1)
    return a.to_broadcast([ap2d.shape[0], ap2d.shape[1], last])


def build(nsh_pad):
    """Emit the bass program for one core processing nsh_pad points."""
    assert nsh_pad % (T * S) == 0
    nt = nsh_pad // T           # tiles
    nst = nt // S               # supertiles
    FW = S * T                  # feature-stage width (896)
    CH = [(0, 512), (512, 384)]

    nc = bacc.Bacc(trn_type="TRN2")

    # ---------------- DRAM tensors ----------------
    featT = nc.dram_tensor("featT", [D_IN, nsh_pad], BF16, kind="ExternalInput")
    geoT = nc.dram_tensor("geoT", [10, K, nsh_pad], BF16, kind="ExternalInput")
    out_d = nc.dram_tensor("out", [nt, 4 * T, T], BF16, kind="ExternalOutput")

    def din(name, shape, dt):
        return nc.dram_tensor(name, shape, dt, kind="ExternalInput")

    wd = {}
    for ln in ("l1", "l2"):
        wd[ln + "_W1a"] = din(ln + "_W1a", [10, 65], BF16)
        wd[ln + "_b1a"] = din(ln + "_b1a", [65, 1], F32)
        wd[ln + "_W2a"] = din(ln + "_W2a", [65, 128], BF16)
    wd["m1_W1"] = din("m1_W1", [128, 128], BF16)
    wd["m1_W2"] = din("m1_W2", [128, 128], BF16)
    wd["m1_b1"] = din("m1_b1", [128, 1], F32)
    wd["m1_b2"] = din("m1_b2", [128, 1], F32)
    wd["p1_W1"] = din("p1_W1", [128, 512], BF16)
    wd["p1_b1"] = din("p1_b1", [128, 2], F32)
    wd["p1_W2"] = din("p1_W2", [128, 256], BF16)
    wd["p1_b2"] = din("p1_b2", [128, 1], F32)
    wd["p2_W1"] = din("p2_W1", [128, 512], BF16)
    wd["p2_b1"] = din("p2_b1", [128, 2], F32)
    wd["p2_W2"] = din("p2_W2", [128, 512], BF16)
    wd["p2_b2"] = din("p2_b2", [128, 2], F32)
    wd["m2_W1"] = din("m2_W1", [128, 768], BF16)     # [256,384]
    wd["m2_b1"] = din("m2_b1", [128, 3], F32)
    wd["m2_W2"] = din("m2_W2", [128, 1536], BF16)    # [384,512]
    wd["m3_W1"] = din("m3_W1", [128, 256], BF16)     # [128,256]
    wd["m3_b1"] = din("m3_b1", [128, 2], F32)
    wd["m3_W2"] = din("m3_W2", [128, 1024], BF16)    # [256,512]
    wd["b2s"] = din("b2s", [128, 4], F32)            # bpack(m2_b2+m3_b2)

    with tile.TileContext(nc) as tc:
        import contextlib
        with contextlib.ExitStack() as ctx:
            cst = ctx.enter_context(tc.tile_pool(name="cst", bufs=1))
            geo_p = ctx.enter_context(tc.tile_pool(name="geo", bufs=7))
            enc_p = ctx.enter_context(tc.tile_pool(name="enc", bufs=4))
            pool_p = ctx.enter_context(tc.tile_pool(name="pool", bufs=2))
            feat_p = ctx.enter_context(tc.tile_pool(name="feat", bufs=2))
            ps_z = ctx.enter_context(
                tc.tile_pool(name="psz", bufs=2, space="PSUM"))
            ps_h = ctx.enter_context(
                tc.tile_pool(name="psh", bufs=1, space="PSUM"))
            ps_m = ctx.enter_context(
                tc.tile_pool(name="psm", bufs=2, space="PSUM"))

            # ---------- constants ----------
            w_sb = {}
            for name, dr in wd.items():
                t_ = cst.tile(list(dr.shape), dr.dtype, name="w_" + name)
                nc.sync.dma_start(out=t_[:], in_=dr[:])
                w_sb[name] = t_
            ident = cst.tile([128, 128], BF16, name="ident")
            make_identity(nc, ident[:])

            def fmlp(rhs_tiles, wkey, bkey, nci, ncout, func, out_t,
                     leaky=False, extra=None, defer=None, chunk_sel=None):
                """Channel-major MLP layer over the supertile.
                rhs_tiles: list of nci tiles [128, FW]; out_t [128, nco_blk, FW].
                extra: (rhs_tiles2, wkey2, nci2) accumulated into same psum.
                defer: if a list, append one emission closure per
                (co, chunk) block instead of emitting now."""
                w = w_sb[wkey]
                b = w_sb[bkey]
                nblk = ncout // 128

                def emit_block(co, c0, cw):
                    ps = ps_m.tile([128, 512], F32, tag="misc", name="ps_f")
                    tot = nci + (extra[2] if extra else 0)
                    i = 0
                    for ci in range(nci):
                        nc.tensor.matmul(
                            out=ps[:, :cw],
                            lhsT=w[:, ci * ncout + co * 128:
                                   ci * ncout + (co + 1) * 128],
                            rhs=rhs_tiles[ci][:, c0:c0 + cw],
                            start=(i == 0),
                            stop=(i == tot - 1))
                        i += 1
                    if extra:
                        rhs2, wkey2, nci2 = extra
                        w2 = w_sb[wkey2]
                        for ci in range(nci2):
                            nc.tensor.matmul(
                                out=ps[:, :cw],
                                lhsT=w2[:, ci * ncout + co * 128:
                                        ci * ncout + (co + 1) * 128],
                                rhs=rhs2[ci][:, c0:c0 + cw],
                                start=(i == 0),
                                stop=(i == tot - 1))
                            i += 1
                    if leaky:
                        nc.scalar.activation(
                            out=out_t[:, co, c0:c0 + cw], in_=ps[:, :cw],
                            func=AF.Prelu, bias=b[:, co:co + 1],
                            alpha=0.01)
                    else:
                        nc.scalar.activation(
                            out=out_t[:, co, c0:c0 + cw], in_=ps[:, :cw],
                            func=func, bias=b[:, co:co + 1])

                for ci_, (c0, cw) in enumerate(CH):
                    if chunk_sel is not None and ci_ not in chunk_sel:
                        continue
                    for co in range(nblk):
                        if defer is None:
                            emit_block(co, c0, cw)
                        else:
                            defer.append(
                                lambda co=co, c0=c0, cw=cw:
                                emit_block(co, c0, cw))

            def fhalf_alloc():
                Ef = pool_p.tile([128, S, T], BF16, tag="Ef", name="Ef")
                Sf = pool_p.tile([128, S], F32, tag="Sf", name="Sf")
                g = pool_p.tile([128, S, T], BF16, tag="g", name="g")
                return None, Ef, Sf, g

            def fhalf_wave(fh, xcm, t0, t1):
                """f-half for tiles [t0, t1) of a round."""
                _, Ef, Sf, g = fh
                w = (t1 - t0) * T
                fTp = ps_m.tile([128, w], BF16, tag="misc", name="fTp")
                for t in range(t0, t1):
                    nc.tensor.transpose(
                        out=fTp[:, (t - t0) * T:(t - t0 + 1) * T],
                        in_=xcm[:, t * T:(t + 1) * T], identity=ident[:])
                nc.scalar.activation(
                    out=Ef[:, t0:t1, :].rearrange("p s t -> p (s t)"),
                    in_=fTp[:], func=AF.Exp)
                nc.vector.reduce_sum(out=Sf[:, t0:t1], in_=Ef[:, t0:t1, :],
                                     axis=AX.X)
                nc.vector.tensor_mul(
                    out=g[:, t0:t1, :].rearrange("p s t -> p (s t)"),
                    in0=fTp[:],
                    in1=Ef[:, t0:t1, :].rearrange("p s t -> p (s t)"))

            def enc_geo(geo_t, lname, s_slot):
                """x-independent per-tile encoder work: h, E, M, s."""
                w1a = w_sb[lname + "_W1a"]
                b1a = w_sb[lname + "_b1a"]
                w2a = w_sb[lname + "_W2a"]
                # L1: relu(W1a.T @ geo) -> h [65, K, T] bf16 (row 64 == 1)
                h = enc_p.tile([65, K, T], BF16, tag="h", name="h")
                for c in range(2):
                    ph = ps_h.tile([65, 8, T], F32, tag="ph", name="ph")
                    for c2 in range(2):
                        nc.tensor.matmul(
                            out=ph[:, 4 * c2:4 * c2 + 4, :], lhsT=w1a[:],
                            rhs=geo_t[0:10, 8 * c + 4 * c2:
                                      8 * c + 4 * c2 + 4, :])
                    nc.scalar.activation(
                        out=h[:, 8 * c:8 * c + 8, :], in_=ph[:],
                        func=AF.Relu, bias=b1a[:, 0:1])
                # L2 row-major: z_k = h_k.T @ W2a -> psum [pt, c]
                zA = ps_z.tile([128, 8, T], F32, tag="z", name="zA")
                zB = ps_z.tile([128, 8, T], F32, tag="z", name="zB")
                for k in range(K):
                    zt = zA if k < 8 else zB
                    nc.tensor.matmul(
                        out=zt[:, k % 8, :], lhsT=h[:, k, :], rhs=w2a[:])
                E = enc_p.tile([128, K, T], BF16, tag="E", name="E", bufs=2)
                M = enc_p.tile([128, K, T], BF16, tag="M", name="M", bufs=8)
                nc.scalar.activation(out=E[:, 0:8, :], in_=zA[:], func=AF.Exp)
                nc.vector.tensor_mul(out=M[:, 0:8, :], in0=E[:, 0:8, :],
                                     in1=zA[:])
                nc.scalar.activation(out=E[:, 8:16, :], in_=zB[:], func=AF.Exp)
                nc.vector.tensor_mul(out=M[:, 8:16, :], in0=E[:, 8:16, :],
                                     in1=zB[:])
                # s[pt, k] = sum_c E -> batched per-round tile slot
                with nc.allow_low_precision("bf16 softmax denom"):
                    nc.vector.reduce_sum(out=s_slot, in_=E[:], axis=AX.X)
                return M

            def wave_r(s_all, Sf_all, g_all, t0, t1):
                """Batched r/sr/gsr for tiles [t0, t1): r_all slice + gsr."""
                w = t1 - t0
                r_all = pool_p.tile([128, S, K], F32, tag="r", name="r_all",
                                    bufs=2) if t0 == 0 else None
                if t0 == 0:
                    wave_r.cur = r_all
                r_all = wave_r.cur
                den = r_all[:, t0:t1, :]
                nc.vector.tensor_tensor(
                    out=den, in0=s_all[:, t0:t1, :],
                    in1=_bcast_last(Sf_all[:, t0:t1], K), op=ALU.add)
                nc.vector.reciprocal(
                    out=den.rearrange("p s k -> p (s k)"),
                    in_=den.rearrange("p s k -> p (s k)"))
                sr_w = pool_p.tile([128, S], F32, tag="sr", name="sr_w",
                                   bufs=2) if t0 == 0 else None
                if t0 == 0:
                    wave_r.sr = sr_w
                sr_w = wave_r.sr
                nc.vector.reduce_sum(out=sr_w[:, t0:t1],
                                     in_=r_all[:, t0:t1, :], axis=AX.X)
                gsr = pool_p.tile([128, S, T], BF16, tag="gsr", name="gsr",
                                  bufs=2) if t0 == 0 else None
                if t0 == 0:
                    wave_r.gsr = gsr
                gsr = wave_r.gsr
                nc.gpsimd.tensor_tensor(
                    out=gsr[:, t0:t1, :], in0=g_all[:, t0:t1, :],
                    in1=_bcast_last(sr_w[:, t0:t1], T), op=ALU.mult)
                return r_all, gsr

            def enc_tail(M, r_all, t_in_s):
                """x-dependent pool tail for one tile (enc half)."""
                # ctb = M * r_k (gpsimd); k-sum via tree adds
                ctb = enc_p.tile([128, K, T], BF16, tag="ct", name="ctb")
                nc.gpsimd.tensor_tensor(
                    out=ctb[:], in0=M[:],
                    in1=_bcast_last(r_all[:, t_in_s, :], T),
                    op=ALU.mult)
                pr = pool_p.tile([128, T], BF16, tag="pr", name="pr",
                                 bufs=5)
                nc.vector.tensor_add(
                    out=ctb[:, 0:8, :], in0=ctb[:, 0:8, :],
                    in1=ctb[:, 8:16, :])
                for hw_ in (4, 2):
                    nc.gpsimd.tensor_add(
                        out=ctb[:, 0:hw_, :], in0=ctb[:, 0:hw_, :],
                        in1=ctb[:, hw_:2 * hw_, :])
                nc.gpsimd.tensor_add(out=pr[:], in0=ctb[:, 0, :],
                                     in1=ctb[:, 1, :])
                return pr

            def pool_transpose(pr, gsr, t_in_s, pcm):
                # transpose pool rows back to channel-major (deferred so it
                # does not gate the next round's PE work); one fused copy
                c0 = t_in_s * T
                pt2 = ps_m.tile([128, 2 * T], BF16, tag="misc", name="pt2")
                nc.tensor.transpose(
                    out=pt2[:, 0:T], in_=pr[:], identity=ident[:])
                nc.tensor.transpose(
                    out=pt2[:, T:2 * T], in_=gsr[:, t_in_s, :],
                    identity=ident[:])
                nc.scalar.copy(
                    out=pcm[:, :, c0:c0 + T],
                    in_=pt2[:].rearrange("p (h t) -> p h t", h=2))

            # ================= main loop (wave-pipelined rounds) ====
            # Rounds r: even = enc1(st), odd = enc2(st). Each round runs in
            # two waves (tiles 0-3, 4-6). A wave's pool results feed the
            # p-MLP chunk for that wave immediately, so the next round's
            # f-half (and hence its tails) start before this round ends.
            pending = []

            def prelude(st):
                # ft DMA + m1 MLP for supertile st -> (ft, x1)
                g0 = st * S
                ft = feat_p.tile([128, 1, FW], BF16, tag="ft", name="ft",
                                 bufs=3)
                nc.sync.dma_start(
                    out=ft[:, 0, :], in_=featT[:, g0 * T:(g0 + S) * T])
                m1h = feat_p.tile([128, 1, FW], BF16, tag="m1h", name="m1h", bufs=1)
                fmlp([ft[:, 0, :]], "m1_W1", "m1_b1", 1, 128, AF.Relu, m1h)
                x1 = feat_p.tile([128, 1, FW], BF16, tag="x1", name="x1")
                fmlp([m1h[:, 0, :]], "m1_W2", "m1_b2", 1, 128, AF.Identity,
                     x1)
                return ft, x1[:, 0, :]

            nrounds = 2 * nst
            all_geos = {}

            def geo_dma(st, t):
                g = st * S + t
                geo_t = geo_p.tile([10, K, T], BF16, tag="geo", name="geo_t")
                nc.sync.dma_start(out=geo_t[:],
                                  in_=geoT[:, :, g * T:(g + 1) * T])
                all_geos[(st, t)] = geo_t

            def geo_side(r, t, s_all):
                st, ln = r // 2, ("l1" if r % 2 == 0 else "l2")
                if ln == "l1":
                    geo_dma(st, t)
                return enc_geo(all_geos[(st, t)], ln, s_all[:, t, :])

            WAVES = [(0, 4), (4, S)]
            ft_x1 = prelude(0)
            s_cur = pool_p.tile([128, S, K], BF16, tag="sal", name="s_cur",
                                bufs=2)
            cur = [geo_side(0, t, s_cur) for t in range(S)]
            fh = fhalf_alloc()
            rg = [None, None]
            for wi, (t0, t1) in enumerate(WAVES):
                fhalf_wave(fh, ft_x1[1], t0, t1)
                rg[wi] = wave_r(s_cur, fh[2], fh[3], t0, t1)
            ctx_st = {}
            for r in range(nrounds):
                st = r // 2
                even = (r % 2 == 0)
                if even:
                    ft, x1 = ft_x1
                    ctx_st = {"ft": ft}
                    pcm = feat_p.tile([128, 2, FW], BF16, tag="p1cm",
                                      name="pcm1")
                else:
                    pcm = feat_p.tile([128, 2, FW], BF16, tag="p2cm",
                                      name="pcm2")
                fhn = None
                prs = []
                nxt = []
                rg_cur, rg = rg, [None, None]
                if r + 1 < nrounds:
                    s_cur = pool_p.tile([128, S, K], BF16, tag="sal",
                                        name="s_cur", bufs=2)
                for wi, (t0, t1) in enumerate(WAVES):
                    r_all, gsr = rg_cur[wi]
                    for t in range(t0, t1):
                        prs.append(enc_tail(cur[t], r_all, t))
                        if r + 1 < nrounds:
                            nxt.append(geo_side(r + 1, t, s_cur))
                        if pending:
                            pending.pop(0)()
                        if pending:
                            pending.pop(0)()
                    for t in range(t0, t1):
                        pool_transpose(prs[t], gsr, t, pcm)
                    # p-MLP chunk for this wave -> x_{r+1} chunk
                    if even:
                        if wi == 0:
                            p1h = feat_p.tile([128, 2, FW], BF16, tag="p1h",
                                              name="p1h")
                            x2t = feat_p.tile([128, 1, FW], BF16, tag="x2",
                                              name="x2t")
                        fmlp([pcm[:, 0, :], pcm[:, 1, :]], "p1_W1", "p1_b1",
                             2, 256, AF.Relu, p1h, chunk_sel=[wi])
                        fmlp([p1h[:, 0, :], p1h[:, 1, :]], "p1_W2", "p1_b2",
                             2, 128, AF.Identity, x2t, chunk_sel=[wi])
                        xn = x2t[:, 0, :]
                    else:
                        if wi == 0:
                            p2h = feat_p.tile([128, 2, FW], BF16, tag="p2h",
                                              name="p2h")
                            x3 = feat_p.tile([128, 2, FW], BF16, tag="x3",
                                             name="x3")
                        fmlp([pcm[:, 0, :], pcm[:, 1, :]], "p2_W1", "p2_b1",
                             2, 256, AF.Relu, p2h, chunk_sel=[wi])
                        fmlp([p2h[:, 0, :], p2h[:, 1, :]], "p2_W2", "p2_b2",
                             2, 256, AF.Identity, x3, chunk_sel=[wi])
                        xn = ctx_st.get("x1n")
                    # next round's f-half + r-chain for this wave
                    if r + 1 < nrounds:
                        if wi == 0:
                            fhn = fhalf_alloc()
                        fhalf_wave(fhn, xn, t0, t1)
                        rg[wi] = wave_r(s_cur, fhn[2], fhn[3], t0, t1)
                cur = nxt
                fh = fhn
                if even:
                    ctx_st["x2"] = x2t[:, 0, :]
                    if st + 1 < nst:
                        ft_x1 = prelude(st + 1)
                        ctx_st["x1n"] = ft_x1[1]
                else:
                    g0 = st * S
                    m2h = feat_p.tile([128, 3, FW], BF16, tag="m2h",
                                      name="m2h")
                    fmlp([x3[:, 0, :], x3[:, 1, :]], "m2_W1", "m2_b1", 2,
                         384, AF.Relu, m2h, defer=pending)
                    m3h = feat_p.tile([128, 2, FW], BF16, tag="m3h",
                                      name="m3h")
                    fmlp([ctx_st["ft"][:, 0, :]], "m3_W1", "m3_b1", 1, 256,
                         AF.Relu, m3h, defer=pending)
                    osb = feat_p.tile([128, 4, FW], BF16, tag="osb",
                                      name="osb")
                    fmlp([m2h[:, 0, :], m2h[:, 1, :], m2h[:, 2, :]],
                         "m2_W2", "b2s", 3, 512, None, osb, leaky=True,
                         extra=([m3h[:, 0, :], m3h[:, 1, :]], "m3_W2", 2),
                         defer=pending)

                    def emit_out(osb=osb, g0=g0):
                        for b in range(4):
                            nc.sync.dma_start(
                                out=out_d[g0:g0 + S, b * T:(b + 1) * T, :]
                                .rearrange("t c p -> c t p"),
                                in_=osb[:, b, :].rearrange(
                                    "c (t p) -> c t p", t=S))
                    pending.append(emit_out)
            while pending:
                pending.pop(0)()

    nc.finalize()
    return nc


_BUILD_CACHE = {}


def _get_prog(nsh_pad):
    if nsh_pad not in _BUILD_CACHE:
        _BUILD_CACHE[nsh_pad] = build(nsh_pad)
    return _BUILD_CACHE[nsh_pad]


def _prep_weights(i):
    """Host-side weight packing -> dict of arrays (shared across cores)."""
    o = {}

    def blkpack(W, cout):
        cin = W.shape[0]
        nci = cin // 128
        return np.concatenate([W[ci * 128:(ci + 1) * 128, :]
                               for ci in range(nci)], axis=1)

    def bpack(b):
        nblk = b.shape[0] // 128
        return np.ascontiguousarray(b.reshape(nblk, 128).T)

    for ln in ("l1", "l2"):
        W1, b1 = i[ln + "_W1"], i[ln + "_b1"]
        W2, b2 = i[ln + "_W2"], i[ln + "_b2"]
        W1a = np.concatenate([W1, np.zeros((10, 1), np.float32)], 1)
        W1a = W1a[[6, 7, 8, 9, 0, 1, 2, 3, 4, 5], :]   # [rel,dist,orig,nbr]
        o[ln + "_W1a"] = W1a.astype(BF)
        o[ln + "_b1a"] = np.concatenate(
            [b1, np.ones(1, np.float32)]).reshape(65, 1).astype(np.float32)
        o[ln + "_W2a"] = np.concatenate(
            [W2, b2[None, :]], 0).astype(BF)
    o["m1_W1"] = i["m1_W1"].astype(BF)
    o["m1_W2"] = i["m1_W2"].astype(BF)
    o["m1_b1"] = i["m1_b1"].reshape(128, 1).astype(np.float32)
    o["m1_b2"] = i["m1_b2"].reshape(128, 1).astype(np.float32)
    o["p1_W1"] = blkpack(i["p1_W1"], 256).astype(BF)
    o["p1_b1"] = bpack(i["p1_b1"]).astype(np.float32)
    o["p1_W2"] = blkpack(i["p1_W2"], 128).astype(BF)
    o["p1_b2"] = i["p1_b2"].reshape(128, 1).astype(np.float32)
    o["p2_W1"] = blkpack(i["p2_W1"], 256).astype(BF)
    o["p2_b1"] = bpack(i["p2_b1"]).astype(np.float32)
    o["p2_W2"] = blkpack(i["p2_W2"], 256).astype(BF)
    o["p2_b2"] = bpack(i["p2_b2"]).astype(np.float32)
    o["m2_W1"] = blkpack(i["m2_W1"], 384).astype(BF)
    o["m2_b1"] = bpack(i["m2_b1"]).astype(np.float32)
    o["m2_W2"] = blkpack(i["m2_W2"], 512).astype(BF)
    o["m3_W1"] = blkpack(i["m3_W1"], 256).astype(BF)
    o["m3_b1"] = bpack(i["m3_b1"]).astype(np.float32)
    o["m3_W2"] = blkpack(i["m3_W2"], 512).astype(BF)
    o["b2s"] = bpack(i["m2_b2"] + i["m3_b2"]).astype(np.float32)
    return o


def _prep_core(coords, features, neighbor_idx, c0, c1, nsh_pad):
    nsh = c1 - c0
    pad = nsh_pad - nsh
    feat = features[c0:c1]
    if pad:
        feat = np.concatenate(
            [feat, np.zeros((pad, feat.shape[1]), np.float32)], 0)
    featT = np.ascontiguousarray(feat.T).astype(BF)
    # geometry rows: [rel(3), dist(1), orig(3), nbr(3)] -> [10, K, nsh_pad]
    cs = coords[c0:c1]                               # (nsh, 3)
    nbr = coords[neighbor_idx[c0:c1]]                # (nsh, K, 3)
    orig = np.broadcast_to(cs[:, None, :], nbr.shape)
    rel = orig - nbr
    dist = np.sqrt(np.sum(rel * rel, axis=-1, keepdims=True))
    geo = np.concatenate([rel, dist, orig, nbr], axis=-1)  # (nsh, K, 10)
    if pad:
        geo = np.concatenate(
            [geo, np.zeros((pad, K, 10), np.float32)], 0)
    geoT = np.ascontiguousarray(np.transpose(geo, (2, 1, 0))).astype(BF)
    return {"featT": featT, "geoT": geoT}


def prepare_in_maps(inputs, nsh_pad):
    coords = np.asarray(inputs["coords"], np.float32)
    features = np.asarray(inputs["features"], np.float32)
    idx = np.asarray(inputs["neighbor_idx"])
    wmaps = _prep_weights({k: np.asarray(v, np.float32)
                           for k, v in inputs.items()
                           if k not in ("coords", "features", "neighbor_idx")})
    in_maps = []
    for c in range(NCORES):
        m = dict(wmaps)
        m.update(_prep_core(coords, features, idx,
                            c * NSH, (c + 1) * NSH, nsh_pad))
        in_maps.append(m)
    return in_maps


def assemble_out(results, nsh_pad):
    outs = []
    for c in range(NCORES):
        r = np.asarray(results[c]["out"]).astype(np.float32)  # [nt,512,T]
        r = np.transpose(r, (0, 2, 1)).reshape(nsh_pad, 512)[:NSH]
        outs.append(r)
    return np.ascontiguousarray(np.concatenate(outs, 0))


LAST_RES = None


def kernel(**inputs):
    global LAST_RES
    nsh_pad = _ceil_to(NSH, T * S)               # 12544
    nc = _get_prog(nsh_pad)
    in_maps = prepare_in_maps(inputs, nsh_pad)
    trace = bool(os.environ.get("KERNEL_TRACE"))
    res = run_bass_kernel_spmd(nc, in_maps, core_ids=list(range(NCORES)),
                               trace=trace)
    LAST_RES = res
    return assemble_out(res.results, nsh_pad)


if __name__ == "__main__":
    rng = np.random.default_rng(0)
    inp = {
        "coords": rng.standard_normal((N_FULL, 3)).astype(np.float32),
        "features": rng.standard_normal((N_FULL, 128)).astype(np.float32),
        "neighbor_idx": rng.integers(0, N_FULL, (N_FULL, 16), dtype=np.int32),
    }
    for nm, ci, ch, co in [("m1", 128, 128, 128), ("m2", 256, 384, 512),
                           ("m3", 128, 256, 512), ("l1", 10, 64, 128),
                           ("l2", 10, 64, 128), ("p1", 256, 256, 128),
                           ("p2", 256, 256, 256)]:
        inp[nm + "_W1"] = rng.standard_normal((ci, ch)).astype(np.float32)
        inp[nm + "_b1"] = rng.standard_normal(ch).astype(np.float32)
        inp[nm + "_W2"] = rng.standard_normal((ch, co)).astype(np.float32)
        inp[nm + "_b2"] = rng.standard_normal(co).astype(np.float32)
    out = kernel(**inp)
    print("out", out.shape, out.dtype)
